# revision 25
# baseline (speedup 1.0000x reference)
"""Trainium2 Bass kernel for nn_AdaMus loss_fn (multi-view encoder + pairwise loss).

Strategy: data-parallel over batch (512 rows/core on 8 cores).
 - Activations stored transposed [feature, rows]; all matmuls bf16 with f32 PSUM.
 - Sync-BatchNorm: per-feature sum/sumsq via fused ACT accum, AllReduce'd.
 - fusion (bf16 + its column sumsq row) AllGather'd; pairwise dist row-block
   computed entirely on TensorE via an augmented matmul; loss elementwise on
   DVE/ACT with fused row reductions. Diagonal excluded by setting diag(S)=1
   host-side (constant structural mask) so diag contributes ~0.
 - Outputs per core: fusionT [512,512] f32 + partial loss [1,1]; host gathers.
"""
import sys
if '/opt/trn_rl_repo' not in sys.path:
    sys.path.insert(0, '/opt/trn_rl_repo')

import numpy as np
import concourse.bass as bass
import concourse.mybir as mybir
import concourse.tile as tile
from concourse import bacc
from concourse.bass_utils import run_bass_kernel_spmd

F32 = mybir.dt.float32
F16 = mybir.dt.float16
BF16 = mybir.dt.bfloat16
AF = mybir.ActivationFunctionType
OP = mybir.AluOpType

NC = 8
B = 4096
RPC = B // NC            # 512 rows per core
K0, K1, K2 = 5000, 2000, 1000
K0P, K1P, K2P = 5120, 2048, 1024    # padded to x128 for clean chunking
HD = 1024                # rd_net out
DH = 2048                # encoder hidden
CD = 512                 # com_dim
BN_EPS = 1e-5
LAMBDA1 = 0.01
INV_B = 1.0 / B
SC_G = -2.0 * (INV_B * INV_B)    # -2/B^2, exact power of two
SC_SQ = INV_B * INV_B            # 1/B^2, exact power of two
GSCALE = LAMBDA1 / (CD * NC)     # per-core gamma-L1 term scale

LAST_RESULT = None
_CACHED = {}


def _kchunks(K):
    return [(s, min(128, K - s)) for s in range(0, K, 128)]


def _groups(chs, maxn=8):
    """Group consecutive full chunks into runs of <= maxn; partial chunk alone."""
    out, cur = [], []
    for (s, n) in chs:
        if n == 128 and len(cur) < maxn:
            cur.append((s, n))
        elif n == 128:
            out.append(cur)
            cur = [(s, n)]
        else:
            if cur:
                out.append(cur)
                cur = []
            out.append([(s, n)])
    if cur:
        out.append(cur)
    return out


def _build():
    import os
    STAGE = int(os.environ.get("KSTAGE", "7"))
    nc = bacc.Bacc(None, target_bir_lowering=False, debug=False)

    # ---- I/O declarations -------------------------------------------------
    x0t = nc.dram_tensor("x0t", [K0P, RPC], F32, kind="ExternalInput")
    x1t = nc.dram_tensor("x1t", [K1P, RPC], F32, kind="ExternalInput")
    x2t = nc.dram_tensor("x2t", [K2P, RPC], F32, kind="ExternalInput")
    wrdn = nc.dram_tensor("wrdn", [K0P, HD], F32, kind="ExternalInput")
    w1_0 = nc.dram_tensor("w1_0", [DH // 128, 128, HD // 128, 128], F32, kind="ExternalInput")
    w1_1 = nc.dram_tensor("w1_1", [DH // 128, 128, K1P // 128, 128], F32, kind="ExternalInput")
    w1_2 = nc.dram_tensor("w1_2", [DH // 128, 128, K2P // 128, 128], F32, kind="ExternalInput")
    w2a = nc.dram_tensor("w2a", [3 * (CD // 128), 128, DH // 128, 128], F32, kind="ExternalInput")
    brd = nc.dram_tensor("brd", [128, HD // 128], F32, kind="ExternalInput")
    g1a = nc.dram_tensor("g1a", [128, 3 * (DH // 128)], F32, kind="ExternalInput")
    be1a = nc.dram_tensor("be1a", [128, 3 * (DH // 128)], F32, kind="ExternalInput")
    g2a = nc.dram_tensor("g2a", [128, 3 * (CD // 128)], F32, kind="ExternalInput")
    be2a = nc.dram_tensor("be2a", [128, 3 * (CD // 128)], F32, kind="ExternalInput")
    sblk = nc.dram_tensor("sblk", [RPC, B], F32, kind="ExternalInput")
    stblk = nc.dram_tensor("stblk", [RPC, B], F32, kind="ExternalInput")

    fusiont = nc.dram_tensor("fusiont", [CD, RPC], F32, kind="ExternalOutput")
    lossp = nc.dram_tensor("lossp", [1, 1], F32, kind="ExternalOutput")

    NM1 = DH // 128      # 16 m-chunks for L1
    NM2 = CD // 128      # 4 m-chunks for L2
    NMR = HD // 128      # 8 m-chunks for rd
    RG = [list(range(NC))]

    from contextlib import ExitStack
    with tile.TileContext(nc) as tc, ExitStack() as es:
        # persistent pools
        cp = es.enter_context(tc.tile_pool(name="consts", bufs=1))
        stp = es.enter_context(tc.tile_pool(name="stats", bufs=1))
        fp = es.enter_context(tc.tile_pool(name="fus", bufs=4))
        pp = es.enter_context(tc.tile_pool(name="psum", bufs=8, space="PSUM"))
        dp = es.enter_context(tc.tile_pool(name="dram", bufs=1, space="DRAM"))
        # encoder-phase pools (closed before the loss phase to free SBUF)
        enc_es = ExitStack()
        xp = enc_es.enter_context(tc.tile_pool(name="xh", bufs=24))
        wsp = enc_es.enter_context(tc.tile_pool(name="wstage", bufs=3))
        wbp = enc_es.enter_context(tc.tile_pool(name="wbp", bufs=4))
        xsp = enc_es.enter_context(tc.tile_pool(name="xstage", bufs=4))
        yp = enc_es.enter_context(tc.tile_pool(name="ys", bufs=34))
        ybp = enc_es.enter_context(tc.tile_pool(name="ysb", bufs=12))
        scrp = enc_es.enter_context(tc.tile_pool(name="scr", bufs=2))
        hp = enc_es.enter_context(tc.tile_pool(name="hh", bufs=32))
        vp = enc_es.enter_context(tc.tile_pool(name="v0h", bufs=8))

        # ---- constants + warmup collective ---------------------------
        dum = cp.tile([128, 1], F32, name="dum")
        nc.vector.memset(dum[:], 0.0)
        wu_in = dp.tile([128, 1], F32, name="wu_in")
        wu_out = dp.tile([128, 1], F32, addr_space="Shared", name="wu_out")
        nc.gpsimd.dma_start(wu_in[:], dum[:])
        nc.gpsimd.collective_compute(
            "AllReduce", OP.add, replica_groups=RG,
            ins=[wu_in[:].opt()], outs=[wu_out[:].opt()])

        brd_t = cp.tile([128, NMR], F32, name="brd_t")
        nc.sync.dma_start(brd_t[:], brd[:])
        g1_t = cp.tile([128, 3 * NM1], F32, name="g1_t")
        nc.sync.dma_start(g1_t[:], g1a[:])
        be1_t = cp.tile([128, 3 * NM1], F32, name="be1_t")
        nc.sync.dma_start(be1_t[:], be1a[:])
        g2_t = cp.tile([128, 3 * NM2], F32, name="g2_t")
        nc.sync.dma_start(g2_t[:], g2a[:])
        be2_t = cp.tile([128, 3 * NM2], F32, name="be2_t")
        nc.sync.dma_start(be2_t[:], be2a[:])
        ones_t = cp.tile([128, 1], BF16, name="ones_t")
        nc.vector.memset(ones_t[:], 1.0)
        onesf_t = cp.tile([128, 1], F32, name="onesf_t")
        nc.vector.memset(onesf_t[:], 1.0)
        acc = cp.tile([128, 5], F32, name="acc")
        nc.vector.memset(acc[:], 0.0)
        accw = cp.tile([128, 64], F32, name="accw")
        nc.vector.memset(accw[:], 0.0)
        eps_t = cp.tile([128, 1], F32, name="eps_t")
        nc.vector.memset(eps_t[:], BN_EPS)

        # ---- helpers --------------------------------------------------
        def stage_x(xdram, K, label):
            """DMA + cast input xT chunks -> list of resident bf16 tiles."""
            tiles = []
            for ci, (s, n) in enumerate(_kchunks(K)):
                st = xsp.tile([n, RPC], F32, tag="xstage", name=f"xs_{label}_{ci}")
                nc.sync.dma_start(st[:], xdram[s:s + n, :])
                xb = xp.tile([n, RPC], BF16, tag="xh", name=f"xh_{label}_{ci}")
                nc.vector.tensor_scalar(xb[:], st[:], 1.0, None, OP.mult)
                tiles.append(xb)
            return tiles

        def mm_layer(wdram, KP, n_mc, rhs_tiles, consume, mc_base=0, label=""):
            """m-chunk outer: one full-width weight slab per mc (fat 4-8KB
            per-partition descriptors, 4-way partition-split across DMA
            queues), cast via tensor_scalar (2x mode), matmul-accumulate."""
            nch = KP // 128
            for mc in range(n_mc):
                ps = pp.tile([128, RPC], F32, tag="psum", name=f"ps_{label}_{mc}")
                ws = wsp.tile([128, nch, 128], F32, tag="wstage",
                              name=f"ws_{label}_{mc}")
                for q in range(4):
                    nc.sync.dma_start(
                        ws[q * 32:(q + 1) * 32, :, :],
                        wdram[mc_base + mc, q * 32:(q + 1) * 32, :, :])
                wb = wbp.tile([128, nch, 128], BF16, tag="wbp",
                              name=f"wb_{label}_{mc}")
                nc.vector.tensor_scalar(wb[:], ws[:], 1.0, None, OP.mult)
                for ci in range(nch):
                    nc.tensor.matmul(ps[:], wb[:, ci, :], rhs_tiles[ci][:],
                                     start=(ci == 0), stop=(ci == nch - 1))
                consume(mc, ps)

        def l1_consume(stats_t, n_mc, label, ypool):
            ytiles = []
            def consume(mc, ps):
                y = ypool.tile([128, RPC], F16, tag="y", name=f"y_{label}_{mc}")
                nc.scalar.activation(y[:], ps[:], AF.Copy,
                                     accum_out=stats_t[:, mc:mc + 1])
                sc = scrp.tile([128, RPC], F32, tag="scr", name=f"sc_{label}_{mc}")
                nc.scalar.activation(sc[:], ps[:], AF.Square,
                                     accum_out=stats_t[:, n_mc + mc:n_mc + mc + 1])
                ytiles.append(y)
            return consume, ytiles

        def allreduce_stats(stats_t, ncols, label):
            ain = dp.tile([128, ncols], F32, name=f"arin_{label}")
            aout = dp.tile([128, ncols], F32, addr_space="Shared",
                           name=f"arout_{label}")
            nc.gpsimd.dma_start(ain[:], stats_t[:])
            nc.gpsimd.collective_compute(
                "AllReduce", OP.add, replica_groups=RG,
                ins=[ain[:].opt()], outs=[aout[:].opt()])
            back = stp.tile([128, ncols], F32, name=f"arb_{label}")
            nc.gpsimd.dma_start(back[:], aout[:])
            return back

        def ab_math(back, n_mc, g_t, be_t, goff, extra_scale, label):
            """a = g*rsqrt(var+eps)*extra, b = (be - m*g*rsqrt)*extra."""
            m = stp.tile([128, n_mc], F32, name=f"m_{label}")
            nc.vector.tensor_scalar(m[:], back[:, 0:n_mc], INV_B, None, OP.mult)
            e2 = stp.tile([128, n_mc], F32, name=f"e2_{label}")
            nc.vector.tensor_scalar(e2[:], back[:, n_mc:2 * n_mc], INV_B, None,
                                    OP.mult)
            var = stp.tile([128, n_mc], F32, name=f"var_{label}")
            nc.vector.tensor_tensor(var[:], m[:], m[:], OP.mult)
            nc.vector.tensor_tensor(var[:], e2[:], var[:], OP.subtract)
            sd = stp.tile([128, n_mc], F32, name=f"sd_{label}")
            nc.scalar.activation(sd[:], var[:], AF.Sqrt, bias=eps_t[:])
            rs = stp.tile([128, n_mc], F32, name=f"rs_{label}")
            nc.vector.reciprocal(rs[:], sd[:])
            a = stp.tile([128, n_mc], F32, name=f"a_{label}")
            nc.vector.tensor_tensor(a[:], g_t[:, goff:goff + n_mc], rs[:], OP.mult)
            bb = stp.tile([128, n_mc], F32, name=f"b_{label}")
            nc.vector.tensor_tensor(bb[:], m[:], a[:], OP.mult)
            nc.vector.tensor_tensor(bb[:], be_t[:, goff:goff + n_mc], bb[:],
                                    OP.subtract)
            if extra_scale != 1.0:
                nc.vector.tensor_scalar(a[:], a[:], extra_scale, None, OP.mult)
                nc.vector.tensor_scalar(bb[:], bb[:], extra_scale, None, OP.mult)
            return a, bb

        def norm_layer(ytiles, a, bb, n_mc, out_dtype, outpool, label):
            outs = []
            for mc in range(n_mc):
                h = outpool.tile([128, RPC], out_dtype,
                                 tag="hh" if out_dtype == BF16 else "zz",
                                 name=f"h_{label}_{mc}")
                nc.scalar.activation(h[:], ytiles[mc][:], AF.Relu,
                                     bias=bb[:, mc:mc + 1], scale=a[:, mc:mc + 1])
                outs.append(h)
            return outs

        # ================= encoder pipeline ===========================
        with nc.named_scope("L1_enc2"):
            xh2 = stage_x(x2t, K2P, "x2")
            st1_2 = stp.tile([128, 2 * NM1], F32, name="st1_2")
            cons, y2t = l1_consume(st1_2, NM1, "e2l1", yp)
            mm_layer(w1_2, K2P, NM1, xh2, cons, label="e2l1")
        ar2 = allreduce_stats(st1_2, 2 * NM1, "e2l1")

        with nc.named_scope("L1_enc1"):
            xh1 = stage_x(x1t, K1P, "x1")
            st1_1 = stp.tile([128, 2 * NM1], F32, name="st1_1")
            cons, y1t = l1_consume(st1_1, NM1, "e1l1", yp)
            mm_layer(w1_1, K1P, NM1, xh1, cons, label="e1l1")
        ar1 = allreduce_stats(st1_1, 2 * NM1, "e1l1")

        # norm enc2 L1 before rd (AR2 lands during L1_enc1)
        a12, b12 = ab_math(ar2, NM1, g1_t, be1_t, 2 * NM1, 1.0, "e2l1")
        hh2 = norm_layer(y2t, a12, b12, NM1, BF16, hp, "h2")

        # ---- rd net: kc-outer, all 8 psum banks live -----------------
        with nc.named_scope("rd"):
            ps_rd = [pp.tile([128, RPC], F32, tag="psum", name=f"psrd_{mc}")
                     for mc in range(NMR)]
            chs0 = _kchunks(K0P)
            for ci, (s, n) in enumerate(chs0):
                st = xsp.tile([n, RPC], F32, tag="xstage", name=f"xs_x0_{ci}")
                nc.sync.dma_start(st[:], x0t[s:s + n, :])
                xb = xp.tile([n, RPC], BF16, tag="xh", name=f"xh_x0_{ci}")
                nc.vector.tensor_scalar(xb[:], st[:], 1.0, None, OP.mult)
                ws = wsp.tile([n, HD], F32, tag="wstage", name=f"ws_rd_{ci}")
                nc.sync.dma_start(ws[0:n // 2, :], wrdn[s:s + n // 2, :])
                nc.sync.dma_start(ws[n // 2:n, :], wrdn[s + n // 2:s + n, :])
                wb = wbp.tile([n, HD], BF16, tag="wbp", name=f"wb_rd_{ci}")
                nc.vector.tensor_scalar(wb[:], ws[:], 1.0, None, OP.mult)
                for mc in range(NMR):
                    nc.tensor.matmul(ps_rd[mc][:], wb[:, mc * 128:(mc + 1) * 128],
                                     xb[:], start=(ci == 0),
                                     stop=(ci == len(chs0) - 1),
                                     skip_group_check=True)
            v0h = []
            for mc in range(NMR):
                v = vp.tile([128, RPC], BF16, tag="v0", name=f"v0h_{mc}")
                nc.scalar.activation(v[:], ps_rd[mc][:], AF.Relu,
                                     bias=brd_t[:, mc:mc + 1])
                v0h.append(v)

        a11, b11 = ab_math(ar1, NM1, g1_t, be1_t, NM1, 1.0, "e1l1")
        hh1 = norm_layer(y1t, a11, b11, NM1, BF16, hp, "h1")

        with nc.named_scope("L1_enc0"):
            st1_0 = stp.tile([128, 2 * NM1], F32, name="st1_0")
            cons, y0t = l1_consume(st1_0, NM1, "e0l1", yp)
            mm_layer(w1_0, HD, NM1, v0h, cons, label="e0l1")
        ar0 = allreduce_stats(st1_0, 2 * NM1, "e0l1")

        with nc.named_scope("L2_enc2"):
            st2_2 = stp.tile([128, 2 * NM2], F32, name="st2_2")
            cons, y2b = l1_consume(st2_2, NM2, "e2l2", ybp)
            mm_layer(w2a, DH, NM2, hh2, cons, mc_base=2 * NM2, label="e2l2")
        br2 = allreduce_stats(st2_2, 2 * NM2, "e2l2")

        with nc.named_scope("L2_enc1"):
            st2_1 = stp.tile([128, 2 * NM2], F32, name="st2_1")
            cons, y1b = l1_consume(st2_1, NM2, "e1l2", ybp)
            mm_layer(w2a, DH, NM2, hh1, cons, mc_base=NM2, label="e1l2")
        br1 = allreduce_stats(st2_1, 2 * NM2, "e1l2")

        a10, b10 = ab_math(ar0, NM1, g1_t, be1_t, 0, 1.0, "e0l1")
        hh0 = norm_layer(y0t, a10, b10, NM1, BF16, hp, "h0")

        with nc.named_scope("L2_enc0"):
            st2_0 = stp.tile([128, 2 * NM2], F32, name="st2_0")
            cons, y0b = l1_consume(st2_0, NM2, "e0l2", ybp)
            mm_layer(w2a, DH, NM2, hh0, cons, mc_base=0, label="e0l2")
        br0 = allreduce_stats(st2_0, 2 * NM2, "e0l2")

        # ---- norms L2 (scaled by 1/3) + fusion -----------------------
        # z2 normalizes straight into the fusion accumulator tiles
        a22, b22 = ab_math(br2, NM2, g2_t, be2_t, 2 * NM2, 1.0 / 3.0, "e2l2")
        fus = []
        for mc in range(NM2):
            f = fp.tile([128, RPC], F32, tag="fus", name=f"fus_{mc}")
            nc.scalar.activation(f[:], y2b[mc][:], AF.Relu,
                                 bias=b22[:, mc:mc + 1], scale=a22[:, mc:mc + 1])
            fus.append(f)
        a21, b21 = ab_math(br1, NM2, g2_t, be2_t, NM2, 1.0 / 3.0, "e1l2")
        z1 = norm_layer(y1b, a21, b21, NM2, F32, fp, "z1")
        for mc in range(NM2):
            nc.vector.tensor_tensor(fus[mc][:], fus[mc][:], z1[mc][:], OP.add)

        # gamma L1 term -> acc[:,4]  (fills the last-AR wait on ACT)
        gjunk = scrp.tile([128, 3 * NM2], F32, tag="gjunk", name="gjunk")
        nc.scalar.activation(gjunk[:], g2_t[:], AF.Abs, scale=GSCALE,
                             accum_out=acc[:, 4:5])

        a20, b20 = ab_math(br0, NM2, g2_t, be2_t, 0, 1.0 / 3.0, "e0l2")
        z0 = norm_layer(y0b, a20, b20, NM2, F32, fp, "z0")
        for mc in range(NM2):
            nc.vector.tensor_tensor(fus[mc][:], fus[mc][:], z0[mc][:], OP.add)
            nc.sync.dma_start(fusiont[mc * 128:(mc + 1) * 128, :], fus[mc][:])

        # ---- fusion bf16 + sq + AllGather ----------------------------
        with nc.named_scope("gather"):
            fh, fsc, fhsq = [], [], []
            for mc in range(NM2):
                h = fp.tile([128, RPC], BF16, tag="fh", name=f"fh_{mc}")
                nc.vector.tensor_copy(h[:], fus[mc][:])
                fh.append(h)
                s = fp.tile([128, RPC], BF16, tag="fsc", name=f"fsc_{mc}")
                nc.vector.tensor_scalar(s[:], fus[mc][:], SC_G, None, OP.mult)
                fsc.append(s)
                q = fp.tile([128, RPC], BF16, tag="fhsq", name=f"fhsq_{mc}")
                nc.vector.tensor_tensor(q[:], h[:], h[:], OP.mult)
                fhsq.append(q)
            psq = pp.tile([1, RPC], F32, tag="psum", name="psq")
            for mc in range(NM2):
                nc.tensor.matmul(psq[:], ones_t[:], fhsq[mc][:],
                                 start=(mc == 0), stop=(mc == NM2 - 1))
            sqbf = fp.tile([1, RPC], BF16, tag="sqbf", bufs=1, name="sqbf")
            nc.scalar.activation(sqbf[:], psq[:], AF.Copy)
            # lhsT extra rows: row0 = 1/B^2 (for sq_j), row1 = sq_i/B^2
            # (engine ops can't start at partition 1; assemble via DMA)
            c0row = cp.tile([1, RPC], BF16, name="c0row")
            nc.vector.memset(c0row[:], SC_SQ)
            sqsc = cp.tile([1, RPC], BF16, name="sqsc")
            nc.vector.tensor_scalar(sqsc[:], sqbf[:], SC_SQ, None, OP.mult)
            onesrow = cp.tile([1, RPC], BF16, name="onesrow")
            nc.vector.memset(onesrow[:], 1.0)
            lh5 = cp.tile([2, RPC], BF16, name="lh5")
            nc.sync.dma_start(lh5[0:1, :], c0row[:])
            nc.sync.dma_start(lh5[1:2, :], sqsc[:])

            agin = dp.tile([CD + 1, RPC], BF16, name="agin")
            agout = dp.tile([NC * (CD + 1), RPC], BF16, addr_space="Shared",
                            name="agout")
            for mc in range(NM2):
                nc.sync.dma_start(agin[mc * 128:(mc + 1) * 128, :], fh[mc][:])
            nc.sync.dma_start(agin[CD:CD + 1, :], sqbf[:])
            if STAGE >= 4:
                nc.gpsimd.collective_compute(
                    "AllGather", OP.bypass, replica_groups=RG,
                    ins=[agin[:].opt()], outs=[agout[:].opt()])

        # encoder tiles are all dead now; free their SBUF for the loss phase
        enc_es.close()
        gp = es.enter_context(tc.tile_pool(name="gt", bufs=8))
        ssp = es.enter_context(tc.tile_pool(name="sstage", bufs=8))
        lp = es.enter_context(tc.tile_pool(name="ltmp", bufs=3))

        # ---- pairwise dist + loss (A-mask JIT) -----------------------
        with nc.named_scope("loss"):
            spair = {}
            for jc in range(NC if STAGE >= 5 else 0):
                gts = []
                for kc in range(NM2):
                    g = gp.tile([128, RPC], BF16, tag="gt", name=f"g_{jc}_{kc}")
                    nc.sync.dma_start(
                        g[:], agout[jc * (CD + 1) + kc * 128:
                                    jc * (CD + 1) + (kc + 1) * 128, :])
                    gts.append(g)
                sqo = gp.tile([2, RPC], BF16, tag="sqo", bufs=2, name=f"sqo_{jc}")
                nc.sync.dma_start(sqo[0:1, :],
                                  agout[jc * (CD + 1) + CD:
                                        jc * (CD + 1) + CD + 1, :])
                nc.sync.dma_start(sqo[1:2, :], onesrow[:])
                for mc in range(NM2):
                    if jc % 2 == 0:
                        s_t = ssp.tile([128, 2 * RPC], F32, tag="sstage",
                                       name=f"s_{jc}_{mc}")
                        nc.sync.dma_start(
                            s_t[:], sblk[mc * 128:(mc + 1) * 128,
                                         jc * RPC:(jc + 2) * RPC])
                        st_t = ssp.tile([128, 2 * RPC], F32, tag="ststage",
                                        name=f"st_{jc}_{mc}")
                        nc.sync.dma_start(
                            st_t[:], stblk[mc * 128:(mc + 1) * 128,
                                           jc * RPC:(jc + 2) * RPC])
                        spair[mc] = (s_t, st_t)
                    s_t, st_t = spair[mc]
                    off = (jc % 2) * RPC
                    mx = lp.tile([128, RPC], F32, tag="mx", name=f"mx_{jc}_{mc}")
                    nc.vector.tensor_tensor(mx[:], s_t[:, off:off + RPC],
                                            st_t[:, off:off + RPC], OP.max)
                    at = lp.tile([128, RPC], F16, tag="at", name=f"at_{jc}_{mc}")
                    nc.vector.tensor_scalar(at[:], mx[:], 0.6, None, OP.is_gt)

                    ps = pp.tile([128, RPC], F32, tag="psum",
                                 name=f"pl_{jc}_{mc}")
                    for kc in range(NM2):
                        nc.tensor.matmul(
                            ps[:], fsc[kc][:, mc * 128:(mc + 1) * 128],
                            gts[kc][:], start=(kc == 0), stop=False)
                    nc.tensor.matmul(ps[:], lh5[:, mc * 128:(mc + 1) * 128],
                                     sqo[:], start=False, stop=True)
                    # pc = clamped dist in bf16 (negatives are ~1e-12 rounding)
                    pc = lp.tile([128, RPC], BF16, tag="pc", name=f"pc_{jc}_{mc}")
                    nc.vector.tensor_scalar(pc[:], ps[:], 0.0, None, OP.max)
                    u = lp.tile([128, RPC], F16, tag="u1",
                                name=f"u_{jc}_{mc}")
                    nc.scalar.activation(u[:], pc[:], AF.Sqrt)
                    nc.scalar.activation(u[:], u[:], AF.Relu, bias=1.0, scale=-1.0)
                    ci2 = (jc * NM2 + mc) * 2
                    if STAGE >= 6:
                        # neg = (1-d)^2 with fused row-sum into accw column
                        neg = lp.tile([128, RPC], F16, tag="neg",
                                      name=f"neg_{jc}_{mc}")
                        nc.scalar.activation(neg[:], u[:], AF.Square,
                                             accum_out=accw[:, ci2:ci2 + 1])
                    if STAGE >= 7:
                        dn = lp.tile([128, RPC], F16, tag="dn",
                                     name=f"dn_{jc}_{mc}")
                        nc.vector.tensor_tensor(dn[:], pc[:], neg[:], OP.subtract)
                        adn = lp.tile([128, RPC], F16, tag="adn",
                                      name=f"adn_{jc}_{mc}")
                        nc.vector.tensor_tensor(adn[:], dn[:], at[:], OP.mult)
                        nc.vector.tensor_reduce(accw[:, ci2 + 1:ci2 + 2], adn[:],
                                                mybir.AxisListType.X, OP.add)

            # finalize: loss = 0.5*sum(accw) + sum(acc[:,4])
            if STAGE >= 6:
                nc.vector.tensor_reduce(acc[:, 0:1], accw[:],
                                        mybir.AxisListType.X, OP.add)
                nc.vector.tensor_scalar(acc[:, 0:4], acc[:, 0:4], 0.5, None,
                                        OP.mult)

            ps5 = pp.tile([1, 5], F32, tag="psum", name="ps5")
            nc.tensor.matmul(ps5[:], onesf_t[:], acc[:], start=True, stop=True)
            rr = cp.tile([1, 1], F32, name="rr")
            nc.vector.tensor_reduce(rr[:], ps5[:], mybir.AxisListType.X, OP.add)
            nc.sync.dma_start(lossp[:], rr[:])

    nc.compile()
    return nc


def _get_nc():
    if "nc" not in _CACHED:
        _CACHED["nc"] = _build()
    return _CACHED["nc"]


def _prep_inmaps(x0, x1, x2, S, params):
    def colblocks(W, nm, kp):
        """-> [nm, 128p, kp//128, 128j], 4KB-contiguous per partition."""
        W = np.asarray(W, dtype=np.float32)
        din, dout = W.shape
        if din < kp:
            W = np.concatenate(
                [W, np.zeros((kp - din, dout), np.float32)], axis=0)
        # [kp,dout] -> [nch,128p,nm,128j] -> [nm,128p,nch,128j]
        return np.ascontiguousarray(
            W.reshape(kp // 128, 128, nm, 128).transpose(2, 1, 0, 3))

    def padk(x, kp):
        x = np.asarray(x, dtype=np.float32)
        if x.shape[0] < kp:
            x = np.concatenate(
                [x, np.zeros((kp - x.shape[0], x.shape[1]), np.float32)], axis=0)
        return np.ascontiguousarray(x)

    def vec128(v, ncol):
        v = np.asarray(v, dtype=np.float32)
        return np.ascontiguousarray(v.reshape(ncol, 128).T)

    p = params
    wrdn = padk(p['W_rd'], K0P)
    kps = [HD, K1P, K2P]
    w1 = [colblocks(p[f'enc{v}']['W1'], DH // 128, kps[v]) for v in range(3)]
    w2a = np.concatenate([colblocks(p[f'enc{v}']['W2'], CD // 128, DH)
                          for v in range(3)], axis=0)
    brd = vec128(p['b_rd'], HD // 128)
    g1a = np.concatenate([vec128(p[f'enc{v}']['g1'], DH // 128)
                          for v in range(3)], axis=1)
    be1a = np.concatenate([vec128(p[f'enc{v}']['be1'], DH // 128)
                           for v in range(3)], axis=1)
    g2a = np.concatenate([vec128(p[f'enc{v}']['g2'], CD // 128)
                          for v in range(3)], axis=1)
    be2a = np.concatenate([vec128(p[f'enc{v}']['be2'], CD // 128)
                           for v in range(3)], axis=1)

    x0 = np.asarray(x0, dtype=np.float32)
    x1 = np.asarray(x1, dtype=np.float32)
    x2 = np.asarray(x2, dtype=np.float32)
    S = np.asarray(S, dtype=np.float32)
    ST = np.ascontiguousarray(S.T)

    in_maps = []
    idx = np.arange(RPC)
    for r in range(NC):
        rows = slice(r * RPC, (r + 1) * RPC)
        sb = np.ascontiguousarray(S[rows, :])
        stb = np.ascontiguousarray(ST[rows, :])
        # diag(S)=1 makes A_ii True so the diagonal contributes ~0 (dist_ii~0)
        sb[idx, r * RPC + idx] = 1.0
        stb[idx, r * RPC + idx] = 1.0
        in_maps.append({
            "x0t": padk(x0[rows].T, K0P),
            "x1t": padk(x1[rows].T, K1P),
            "x2t": padk(x2[rows].T, K2P),
            "wrdn": wrdn, "w1_0": w1[0], "w1_1": w1[1], "w1_2": w1[2],
            "w2a": w2a, "brd": brd, "g1a": g1a, "be1a": be1a,
            "g2a": g2a, "be2a": be2a,
            "sblk": sb, "stblk": stb,
        })
    return in_maps


def kernel(x0, x1, x2, S, batch, params):
    global LAST_RESULT
    nc = _get_nc()
    in_maps = _prep_inmaps(x0, x1, x2, S, params)
    res = run_bass_kernel_spmd(nc, in_maps, list(range(NC)),
                               trace_cores=list(range(NC)))
    LAST_RESULT = res
    fusion = np.concatenate(
        [np.ascontiguousarray(res.results[r]["fusiont"].T) for r in range(NC)],
        axis=0)
    loss = np.float32(sum(float(res.results[r]["lossp"][0, 0])
                          for r in range(NC)))
    return fusion, loss


# revision 26
# speedup vs baseline: 1.3029x; 1.3029x over previous
"""Trainium2 Bass kernel for nn_AdaMus loss_fn (multi-view encoder + pairwise loss).

Strategy: data-parallel over batch (512 rows/core on 8 cores).
 - Activations stored transposed [feature, rows]; all matmuls bf16 with f32 PSUM.
 - Sync-BatchNorm: per-feature sum/sumsq via fused ACT accum, AllReduce'd.
 - fusion (bf16 + its column sumsq row) AllGather'd; pairwise dist row-block
   computed entirely on TensorE via an augmented matmul; loss elementwise on
   DVE/ACT with fused row reductions. Diagonal excluded by setting diag(S)=1
   host-side (constant structural mask) so diag contributes ~0.
 - Outputs per core: fusionT [512,512] f32 + partial loss [1,1]; host gathers.
"""
import sys
if '/opt/trn_rl_repo' not in sys.path:
    sys.path.insert(0, '/opt/trn_rl_repo')

import numpy as np
import concourse.bass as bass
import concourse.mybir as mybir
import concourse.tile as tile
from concourse import bacc
from concourse.bass_utils import run_bass_kernel_spmd

F32 = mybir.dt.float32
F16 = mybir.dt.float16
BF16 = mybir.dt.bfloat16
AF = mybir.ActivationFunctionType
OP = mybir.AluOpType

NC = 8
B = 4096
RPC = B // NC            # 512 rows per core
K0, K1, K2 = 5000, 2000, 1000
K0P, K1P, K2P = 5120, 2048, 1024    # padded to x128 for clean chunking
HD = 1024                # rd_net out
DH = 2048                # encoder hidden
CD = 512                 # com_dim
BN_EPS = 1e-5
LAMBDA1 = 0.01
INV_B = 1.0 / B
SC_G = -2.0 * (INV_B * INV_B)    # -2/B^2, exact power of two
SC_SQ = INV_B * INV_B            # 1/B^2, exact power of two
GSCALE = LAMBDA1 / (CD * NC)     # per-core gamma-L1 term scale

LAST_RESULT = None
_CACHED = {}


def _kchunks(K):
    return [(s, min(128, K - s)) for s in range(0, K, 128)]


def _groups(chs, maxn=8):
    """Group consecutive full chunks into runs of <= maxn; partial chunk alone."""
    out, cur = [], []
    for (s, n) in chs:
        if n == 128 and len(cur) < maxn:
            cur.append((s, n))
        elif n == 128:
            out.append(cur)
            cur = [(s, n)]
        else:
            if cur:
                out.append(cur)
                cur = []
            out.append([(s, n)])
    if cur:
        out.append(cur)
    return out


def _build():
    import os
    STAGE = int(os.environ.get("KSTAGE", "7"))
    nc = bacc.Bacc(None, target_bir_lowering=False, debug=False)

    # ---- I/O declarations -------------------------------------------------
    x0t = nc.dram_tensor("x0t", [K0P, RPC], F32, kind="ExternalInput")
    x1t = nc.dram_tensor("x1t", [K1P, RPC], F32, kind="ExternalInput")
    x2t = nc.dram_tensor("x2t", [K2P, RPC], F32, kind="ExternalInput")
    wrdn = nc.dram_tensor("wrdn", [K0P, HD], F32, kind="ExternalInput")
    w1_0 = nc.dram_tensor("w1_0", [DH // 128, 128, HD // 128, 128], F32, kind="ExternalInput")
    w1_1 = nc.dram_tensor("w1_1", [DH // 128, 128, K1P // 128, 128], F32, kind="ExternalInput")
    w1_2 = nc.dram_tensor("w1_2", [DH // 128, 128, K2P // 128, 128], F32, kind="ExternalInput")
    w2a = nc.dram_tensor("w2a", [3 * (CD // 128), 128, DH // 128, 128], F32, kind="ExternalInput")
    brd = nc.dram_tensor("brd", [128, HD // 128], F32, kind="ExternalInput")
    g1a = nc.dram_tensor("g1a", [128, 3 * (DH // 128)], F32, kind="ExternalInput")
    be1a = nc.dram_tensor("be1a", [128, 3 * (DH // 128)], F32, kind="ExternalInput")
    g2a = nc.dram_tensor("g2a", [128, 3 * (CD // 128)], F32, kind="ExternalInput")
    be2a = nc.dram_tensor("be2a", [128, 3 * (CD // 128)], F32, kind="ExternalInput")
    sblk = nc.dram_tensor("sblk", [RPC, B], F32, kind="ExternalInput")
    stblk = nc.dram_tensor("stblk", [RPC, B], F32, kind="ExternalInput")

    fusiont = nc.dram_tensor("fusiont", [CD, RPC], F32, kind="ExternalOutput")
    lossp = nc.dram_tensor("lossp", [1, 1], F32, kind="ExternalOutput")

    NM1 = DH // 128      # 16 m-chunks for L1
    NM2 = CD // 128      # 4 m-chunks for L2
    NMR = HD // 128      # 8 m-chunks for rd
    RG = [list(range(NC))]

    from contextlib import ExitStack
    with tile.TileContext(nc) as tc, ExitStack() as es:
        # persistent pools
        cp = es.enter_context(tc.tile_pool(name="consts", bufs=1))
        stp = es.enter_context(tc.tile_pool(name="stats", bufs=1))
        fp = es.enter_context(tc.tile_pool(name="fus", bufs=4))
        pp = es.enter_context(tc.tile_pool(name="psum", bufs=8, space="PSUM"))
        dp = es.enter_context(tc.tile_pool(name="dram", bufs=1, space="DRAM"))
        # encoder-phase pools (closed before the loss phase to free SBUF)
        enc_es = ExitStack()
        xp = enc_es.enter_context(tc.tile_pool(name="xh", bufs=24))
        wsp = enc_es.enter_context(tc.tile_pool(name="wstage", bufs=3))
        wbp = enc_es.enter_context(tc.tile_pool(name="wbp", bufs=4))
        xsp = enc_es.enter_context(tc.tile_pool(name="xstage", bufs=4))
        yp = enc_es.enter_context(tc.tile_pool(name="ys", bufs=34))
        ybp = enc_es.enter_context(tc.tile_pool(name="ysb", bufs=12))
        scrp = enc_es.enter_context(tc.tile_pool(name="scr", bufs=2))
        hp = enc_es.enter_context(tc.tile_pool(name="hh", bufs=32))
        vp = enc_es.enter_context(tc.tile_pool(name="v0h", bufs=8))

        # ---- constants + warmup collective ---------------------------
        dum = cp.tile([128, 1], F32, name="dum")
        nc.vector.memset(dum[:], 0.0)
        wu_in = dp.tile([128, 1], F32, name="wu_in")
        wu_out = dp.tile([128, 1], F32, addr_space="Shared", name="wu_out")
        nc.gpsimd.dma_start(wu_in[:], dum[:])
        nc.gpsimd.collective_compute(
            "AllReduce", OP.add, replica_groups=RG,
            ins=[wu_in[:].opt()], outs=[wu_out[:].opt()])

        brd_t = cp.tile([128, NMR], F32, name="brd_t")
        nc.sync.dma_start(brd_t[:], brd[:])
        g1_t = cp.tile([128, 3 * NM1], F32, name="g1_t")
        nc.sync.dma_start(g1_t[:], g1a[:])
        be1_t = cp.tile([128, 3 * NM1], F32, name="be1_t")
        nc.sync.dma_start(be1_t[:], be1a[:])
        g2_t = cp.tile([128, 3 * NM2], F32, name="g2_t")
        nc.sync.dma_start(g2_t[:], g2a[:])
        be2_t = cp.tile([128, 3 * NM2], F32, name="be2_t")
        nc.sync.dma_start(be2_t[:], be2a[:])
        ones_t = cp.tile([128, 1], BF16, name="ones_t")
        nc.vector.memset(ones_t[:], 1.0)
        onesf_t = cp.tile([128, 1], F32, name="onesf_t")
        nc.vector.memset(onesf_t[:], 1.0)
        acc = cp.tile([128, 5], F32, name="acc")
        nc.vector.memset(acc[:], 0.0)
        accw = cp.tile([128, 64], F32, name="accw")
        nc.vector.memset(accw[:], 0.0)
        eps_t = cp.tile([128, 1], F32, name="eps_t")
        nc.vector.memset(eps_t[:], BN_EPS)

        # ---- helpers --------------------------------------------------
        def stage_x(xdram, K, label):
            """DMA + cast input xT chunks -> list of resident bf16 tiles."""
            tiles = []
            for ci, (s, n) in enumerate(_kchunks(K)):
                st = xsp.tile([n, RPC], F32, tag="xstage", name=f"xs_{label}_{ci}")
                nc.sync.dma_start(st[:], xdram[s:s + n, :])
                xb = xp.tile([n, RPC], BF16, tag="xh", name=f"xh_{label}_{ci}")
                nc.vector.tensor_scalar(xb[:], st[:], 1.0, None, OP.mult)
                tiles.append(xb)
            return tiles

        def mm_layer(wdram, KP, n_mc, rhs_tiles, consume, mc_base=0, label=""):
            """m-chunk outer: one full-width weight slab per mc (fat 4-8KB
            per-partition descriptors, 4-way partition-split across DMA
            queues), cast via tensor_scalar (2x mode), matmul-accumulate."""
            nch = KP // 128
            for mc in range(n_mc):
                ps = pp.tile([128, RPC], F32, tag="psum", name=f"ps_{label}_{mc}")
                ws = wsp.tile([128, nch, 128], F32, tag="wstage",
                              name=f"ws_{label}_{mc}")
                qn = min(4, nch)
                cq = nch // qn
                for q in range(qn):
                    nc.sync.dma_start(
                        ws[:, q * cq:(q + 1) * cq, :],
                        wdram[mc_base + mc, :, q * cq:(q + 1) * cq, :])
                wb = wbp.tile([128, nch, 128], BF16, tag="wbp",
                              name=f"wb_{label}_{mc}")
                nc.vector.tensor_scalar(wb[:], ws[:], 1.0, None, OP.mult)
                for ci in range(nch):
                    nc.tensor.matmul(ps[:], wb[:, ci, :], rhs_tiles[ci][:],
                                     start=(ci == 0), stop=(ci == nch - 1))
                consume(mc, ps)

        def l1_consume(stats_t, n_mc, label, ypool):
            ytiles = []
            def consume(mc, ps):
                y = ypool.tile([128, RPC], F16, tag="y", name=f"y_{label}_{mc}")
                nc.scalar.activation(y[:], ps[:], AF.Copy,
                                     accum_out=stats_t[:, mc:mc + 1])
                sc = scrp.tile([128, RPC], F32, tag="scr", name=f"sc_{label}_{mc}")
                nc.scalar.activation(sc[:], ps[:], AF.Square,
                                     accum_out=stats_t[:, n_mc + mc:n_mc + mc + 1])
                ytiles.append(y)
            return consume, ytiles

        def allreduce_stats(stats_t, ncols, label):
            ain = dp.tile([128, ncols], F32, name=f"arin_{label}")
            aout = dp.tile([128, ncols], F32, addr_space="Shared",
                           name=f"arout_{label}")
            nc.gpsimd.dma_start(ain[:], stats_t[:])
            nc.gpsimd.collective_compute(
                "AllReduce", OP.add, replica_groups=RG,
                ins=[ain[:].opt()], outs=[aout[:].opt()])
            back = stp.tile([128, ncols], F32, name=f"arb_{label}")
            nc.gpsimd.dma_start(back[:], aout[:])
            return back

        def ab_math(back, n_mc, g_t, be_t, goff, extra_scale, label):
            """a = g*rsqrt(var+eps)*extra, b = (be - m*g*rsqrt)*extra."""
            m = stp.tile([128, n_mc], F32, name=f"m_{label}")
            nc.vector.tensor_scalar(m[:], back[:, 0:n_mc], INV_B, None, OP.mult)
            e2 = stp.tile([128, n_mc], F32, name=f"e2_{label}")
            nc.vector.tensor_scalar(e2[:], back[:, n_mc:2 * n_mc], INV_B, None,
                                    OP.mult)
            var = stp.tile([128, n_mc], F32, name=f"var_{label}")
            nc.vector.tensor_tensor(var[:], m[:], m[:], OP.mult)
            nc.vector.tensor_tensor(var[:], e2[:], var[:], OP.subtract)
            sd = stp.tile([128, n_mc], F32, name=f"sd_{label}")
            nc.scalar.activation(sd[:], var[:], AF.Sqrt, bias=eps_t[:])
            rs = stp.tile([128, n_mc], F32, name=f"rs_{label}")
            nc.vector.reciprocal(rs[:], sd[:])
            a = stp.tile([128, n_mc], F32, name=f"a_{label}")
            nc.vector.tensor_tensor(a[:], g_t[:, goff:goff + n_mc], rs[:], OP.mult)
            bb = stp.tile([128, n_mc], F32, name=f"b_{label}")
            nc.vector.tensor_tensor(bb[:], m[:], a[:], OP.mult)
            nc.vector.tensor_tensor(bb[:], be_t[:, goff:goff + n_mc], bb[:],
                                    OP.subtract)
            if extra_scale != 1.0:
                nc.vector.tensor_scalar(a[:], a[:], extra_scale, None, OP.mult)
                nc.vector.tensor_scalar(bb[:], bb[:], extra_scale, None, OP.mult)
            return a, bb

        def norm_layer(ytiles, a, bb, n_mc, out_dtype, outpool, label):
            outs = []
            for mc in range(n_mc):
                h = outpool.tile([128, RPC], out_dtype,
                                 tag="hh" if out_dtype == BF16 else "zz",
                                 name=f"h_{label}_{mc}")
                nc.scalar.activation(h[:], ytiles[mc][:], AF.Relu,
                                     bias=bb[:, mc:mc + 1], scale=a[:, mc:mc + 1])
                outs.append(h)
            return outs

        # ================= encoder pipeline ===========================
        with nc.named_scope("L1_enc2"):
            xh2 = stage_x(x2t, K2P, "x2")
            st1_2 = stp.tile([128, 2 * NM1], F32, name="st1_2")
            cons, y2t = l1_consume(st1_2, NM1, "e2l1", yp)
            mm_layer(w1_2, K2P, NM1, xh2, cons, label="e2l1")
        ar2 = allreduce_stats(st1_2, 2 * NM1, "e2l1")

        with nc.named_scope("L1_enc1"):
            xh1 = stage_x(x1t, K1P, "x1")
            st1_1 = stp.tile([128, 2 * NM1], F32, name="st1_1")
            cons, y1t = l1_consume(st1_1, NM1, "e1l1", yp)
            mm_layer(w1_1, K1P, NM1, xh1, cons, label="e1l1")
        ar1 = allreduce_stats(st1_1, 2 * NM1, "e1l1")

        # norm enc2 L1 before rd (AR2 lands during L1_enc1)
        a12, b12 = ab_math(ar2, NM1, g1_t, be1_t, 2 * NM1, 1.0, "e2l1")
        hh2 = norm_layer(y2t, a12, b12, NM1, BF16, hp, "h2")

        # ---- rd net: kc-outer, all 8 psum banks live -----------------
        with nc.named_scope("rd"):
            ps_rd = [pp.tile([128, RPC], F32, tag="psum", name=f"psrd_{mc}")
                     for mc in range(NMR)]
            chs0 = _kchunks(K0P)
            for ci, (s, n) in enumerate(chs0):
                st = xsp.tile([n, RPC], F32, tag="xstage", name=f"xs_x0_{ci}")
                nc.sync.dma_start(st[:], x0t[s:s + n, :])
                xb = xp.tile([n, RPC], BF16, tag="xh", name=f"xh_x0_{ci}")
                nc.vector.tensor_scalar(xb[:], st[:], 1.0, None, OP.mult)
                ws = wsp.tile([n, HD], F32, tag="wstage", name=f"ws_rd_{ci}")
                nc.sync.dma_start(ws[:, 0:HD // 2], wrdn[s:s + n, 0:HD // 2])
                nc.sync.dma_start(ws[:, HD // 2:], wrdn[s:s + n, HD // 2:])
                wb = wbp.tile([n, HD], BF16, tag="wbp", name=f"wb_rd_{ci}")
                nc.vector.tensor_scalar(wb[:], ws[:], 1.0, None, OP.mult)
                for mc in range(NMR):
                    nc.tensor.matmul(ps_rd[mc][:], wb[:, mc * 128:(mc + 1) * 128],
                                     xb[:], start=(ci == 0),
                                     stop=(ci == len(chs0) - 1),
                                     skip_group_check=True)
            v0h = []
            for mc in range(NMR):
                v = vp.tile([128, RPC], BF16, tag="v0", name=f"v0h_{mc}")
                nc.scalar.activation(v[:], ps_rd[mc][:], AF.Relu,
                                     bias=brd_t[:, mc:mc + 1])
                v0h.append(v)

        a11, b11 = ab_math(ar1, NM1, g1_t, be1_t, NM1, 1.0, "e1l1")
        hh1 = norm_layer(y1t, a11, b11, NM1, BF16, hp, "h1")

        with nc.named_scope("L1_enc0"):
            st1_0 = stp.tile([128, 2 * NM1], F32, name="st1_0")
            cons, y0t = l1_consume(st1_0, NM1, "e0l1", yp)
            mm_layer(w1_0, HD, NM1, v0h, cons, label="e0l1")
        ar0 = allreduce_stats(st1_0, 2 * NM1, "e0l1")

        with nc.named_scope("L2_enc2"):
            st2_2 = stp.tile([128, 2 * NM2], F32, name="st2_2")
            cons, y2b = l1_consume(st2_2, NM2, "e2l2", ybp)
            mm_layer(w2a, DH, NM2, hh2, cons, mc_base=2 * NM2, label="e2l2")
        br2 = allreduce_stats(st2_2, 2 * NM2, "e2l2")

        with nc.named_scope("L2_enc1"):
            st2_1 = stp.tile([128, 2 * NM2], F32, name="st2_1")
            cons, y1b = l1_consume(st2_1, NM2, "e1l2", ybp)
            mm_layer(w2a, DH, NM2, hh1, cons, mc_base=NM2, label="e1l2")
        br1 = allreduce_stats(st2_1, 2 * NM2, "e1l2")

        a10, b10 = ab_math(ar0, NM1, g1_t, be1_t, 0, 1.0, "e0l1")
        hh0 = norm_layer(y0t, a10, b10, NM1, BF16, hp, "h0")

        with nc.named_scope("L2_enc0"):
            st2_0 = stp.tile([128, 2 * NM2], F32, name="st2_0")
            cons, y0b = l1_consume(st2_0, NM2, "e0l2", ybp)
            mm_layer(w2a, DH, NM2, hh0, cons, mc_base=0, label="e0l2")
        br0 = allreduce_stats(st2_0, 2 * NM2, "e0l2")

        # ---- norms L2 (scaled by 1/3) + fusion -----------------------
        # z2 normalizes straight into the fusion accumulator tiles
        a22, b22 = ab_math(br2, NM2, g2_t, be2_t, 2 * NM2, 1.0 / 3.0, "e2l2")
        fus = []
        for mc in range(NM2):
            f = fp.tile([128, RPC], F32, tag="fus", name=f"fus_{mc}")
            nc.scalar.activation(f[:], y2b[mc][:], AF.Relu,
                                 bias=b22[:, mc:mc + 1], scale=a22[:, mc:mc + 1])
            fus.append(f)
        a21, b21 = ab_math(br1, NM2, g2_t, be2_t, NM2, 1.0 / 3.0, "e1l2")
        z1 = norm_layer(y1b, a21, b21, NM2, F32, fp, "z1")
        for mc in range(NM2):
            nc.vector.tensor_tensor(fus[mc][:], fus[mc][:], z1[mc][:], OP.add)

        # gamma L1 term -> acc[:,4]  (fills the last-AR wait on ACT)
        gjunk = scrp.tile([128, 3 * NM2], F32, tag="gjunk", name="gjunk")
        nc.scalar.activation(gjunk[:], g2_t[:], AF.Abs, scale=GSCALE,
                             accum_out=acc[:, 4:5])

        a20, b20 = ab_math(br0, NM2, g2_t, be2_t, 0, 1.0 / 3.0, "e0l2")
        z0 = norm_layer(y0b, a20, b20, NM2, F32, fp, "z0")
        for mc in range(NM2):
            nc.vector.tensor_tensor(fus[mc][:], fus[mc][:], z0[mc][:], OP.add)
            nc.sync.dma_start(fusiont[mc * 128:(mc + 1) * 128, :], fus[mc][:])

        # ---- fusion bf16 + sq + AllGather ----------------------------
        with nc.named_scope("gather"):
            fh, fsc, fhsq = [], [], []
            for mc in range(NM2):
                h = fp.tile([128, RPC], BF16, tag="fh", name=f"fh_{mc}")
                nc.vector.tensor_copy(h[:], fus[mc][:])
                fh.append(h)
                s = fp.tile([128, RPC], BF16, tag="fsc", name=f"fsc_{mc}")
                nc.vector.tensor_scalar(s[:], fus[mc][:], SC_G, None, OP.mult)
                fsc.append(s)
                q = fp.tile([128, RPC], BF16, tag="fhsq", name=f"fhsq_{mc}")
                nc.vector.tensor_tensor(q[:], h[:], h[:], OP.mult)
                fhsq.append(q)
            psq = pp.tile([1, RPC], F32, tag="psum", name="psq")
            for mc in range(NM2):
                nc.tensor.matmul(psq[:], ones_t[:], fhsq[mc][:],
                                 start=(mc == 0), stop=(mc == NM2 - 1))
            sqbf = fp.tile([1, RPC], BF16, tag="sqbf", bufs=1, name="sqbf")
            nc.scalar.activation(sqbf[:], psq[:], AF.Copy)
            # lhsT extra rows: row0 = 1/B^2 (for sq_j), row1 = sq_i/B^2
            # (engine ops can't start at partition 1; assemble via DMA)
            c0row = cp.tile([1, RPC], BF16, name="c0row")
            nc.vector.memset(c0row[:], SC_SQ)
            sqsc = cp.tile([1, RPC], BF16, name="sqsc")
            nc.vector.tensor_scalar(sqsc[:], sqbf[:], SC_SQ, None, OP.mult)
            onesrow = cp.tile([1, RPC], BF16, name="onesrow")
            nc.vector.memset(onesrow[:], 1.0)
            lh5 = cp.tile([2, RPC], BF16, name="lh5")
            nc.sync.dma_start(lh5[0:1, :], c0row[:])
            nc.sync.dma_start(lh5[1:2, :], sqsc[:])

            agin = dp.tile([CD + 1, RPC], BF16, name="agin")
            agout = dp.tile([NC * (CD + 1), RPC], BF16, addr_space="Shared",
                            name="agout")
            for mc in range(NM2):
                nc.sync.dma_start(agin[mc * 128:(mc + 1) * 128, :], fh[mc][:])
            nc.sync.dma_start(agin[CD:CD + 1, :], sqbf[:])
            if STAGE >= 4:
                nc.gpsimd.collective_compute(
                    "AllGather", OP.bypass, replica_groups=RG,
                    ins=[agin[:].opt()], outs=[agout[:].opt()])

        # encoder tiles are all dead now; free their SBUF for the loss phase
        enc_es.close()
        gp = es.enter_context(tc.tile_pool(name="gt", bufs=8))
        ssp = es.enter_context(tc.tile_pool(name="sstage", bufs=8))
        lp = es.enter_context(tc.tile_pool(name="ltmp", bufs=3))

        # ---- pairwise dist + loss (A-mask JIT) -----------------------
        with nc.named_scope("loss"):
            spair = {}
            for jc in range(NC if STAGE >= 5 else 0):
                gts = []
                for kc in range(NM2):
                    g = gp.tile([128, RPC], BF16, tag="gt", name=f"g_{jc}_{kc}")
                    nc.sync.dma_start(
                        g[:], agout[jc * (CD + 1) + kc * 128:
                                    jc * (CD + 1) + (kc + 1) * 128, :])
                    gts.append(g)
                sqo = gp.tile([2, RPC], BF16, tag="sqo", bufs=2, name=f"sqo_{jc}")
                nc.sync.dma_start(sqo[0:1, :],
                                  agout[jc * (CD + 1) + CD:
                                        jc * (CD + 1) + CD + 1, :])
                nc.sync.dma_start(sqo[1:2, :], onesrow[:])
                for mc in range(NM2):
                    if jc % 2 == 0:
                        s_t = ssp.tile([128, 2 * RPC], F32, tag="sstage",
                                       name=f"s_{jc}_{mc}")
                        nc.sync.dma_start(
                            s_t[:], sblk[mc * 128:(mc + 1) * 128,
                                         jc * RPC:(jc + 2) * RPC])
                        st_t = ssp.tile([128, 2 * RPC], F32, tag="ststage",
                                        name=f"st_{jc}_{mc}")
                        nc.sync.dma_start(
                            st_t[:], stblk[mc * 128:(mc + 1) * 128,
                                           jc * RPC:(jc + 2) * RPC])
                        spair[mc] = (s_t, st_t)
                    s_t, st_t = spair[mc]
                    off = (jc % 2) * RPC
                    mx = lp.tile([128, RPC], F32, tag="mx", name=f"mx_{jc}_{mc}")
                    nc.vector.tensor_tensor(mx[:], s_t[:, off:off + RPC],
                                            st_t[:, off:off + RPC], OP.max)
                    at = lp.tile([128, RPC], F16, tag="at", name=f"at_{jc}_{mc}")
                    nc.vector.tensor_scalar(at[:], mx[:], 0.6, None, OP.is_gt)

                    ps = pp.tile([128, RPC], F32, tag="psum",
                                 name=f"pl_{jc}_{mc}")
                    for kc in range(NM2):
                        nc.tensor.matmul(
                            ps[:], fsc[kc][:, mc * 128:(mc + 1) * 128],
                            gts[kc][:], start=(kc == 0), stop=False)
                    nc.tensor.matmul(ps[:], lh5[:, mc * 128:(mc + 1) * 128],
                                     sqo[:], start=False, stop=True)
                    # pc = clamped dist in bf16 (negatives are ~1e-12 rounding)
                    pc = lp.tile([128, RPC], BF16, tag="pc", name=f"pc_{jc}_{mc}")
                    nc.vector.tensor_scalar(pc[:], ps[:], 0.0, None, OP.max)
                    u = lp.tile([128, RPC], F16, tag="u1",
                                name=f"u_{jc}_{mc}")
                    nc.scalar.activation(u[:], pc[:], AF.Sqrt)
                    nc.scalar.activation(u[:], u[:], AF.Relu, bias=1.0, scale=-1.0)
                    ci2 = (jc * NM2 + mc) * 2
                    if STAGE >= 6:
                        # neg = (1-d)^2 with fused row-sum into accw column
                        neg = lp.tile([128, RPC], F16, tag="neg",
                                      name=f"neg_{jc}_{mc}")
                        nc.scalar.activation(neg[:], u[:], AF.Square,
                                             accum_out=accw[:, ci2:ci2 + 1])
                    if STAGE >= 7:
                        dn = lp.tile([128, RPC], F16, tag="dn",
                                     name=f"dn_{jc}_{mc}")
                        nc.vector.tensor_tensor(dn[:], pc[:], neg[:], OP.subtract)
                        adn = lp.tile([128, RPC], F16, tag="adn",
                                      name=f"adn_{jc}_{mc}")
                        nc.vector.tensor_tensor(adn[:], dn[:], at[:], OP.mult)
                        nc.vector.tensor_reduce(accw[:, ci2 + 1:ci2 + 2], adn[:],
                                                mybir.AxisListType.X, OP.add)

            # finalize: loss = 0.5*sum(accw) + sum(acc[:,4])
            if STAGE >= 6:
                nc.vector.tensor_reduce(acc[:, 0:1], accw[:],
                                        mybir.AxisListType.X, OP.add)
                nc.vector.tensor_scalar(acc[:, 0:4], acc[:, 0:4], 0.5, None,
                                        OP.mult)

            ps5 = pp.tile([1, 5], F32, tag="psum", name="ps5")
            nc.tensor.matmul(ps5[:], onesf_t[:], acc[:], start=True, stop=True)
            rr = cp.tile([1, 1], F32, name="rr")
            nc.vector.tensor_reduce(rr[:], ps5[:], mybir.AxisListType.X, OP.add)
            nc.sync.dma_start(lossp[:], rr[:])

    nc.compile()
    return nc


def _get_nc():
    if "nc" not in _CACHED:
        _CACHED["nc"] = _build()
    return _CACHED["nc"]


def _prep_inmaps(x0, x1, x2, S, params):
    def colblocks(W, nm, kp):
        """-> [nm, 128p, kp//128, 128j], 4KB-contiguous per partition."""
        W = np.asarray(W, dtype=np.float32)
        din, dout = W.shape
        if din < kp:
            W = np.concatenate(
                [W, np.zeros((kp - din, dout), np.float32)], axis=0)
        # [kp,dout] -> [nch,128p,nm,128j] -> [nm,128p,nch,128j]
        return np.ascontiguousarray(
            W.reshape(kp // 128, 128, nm, 128).transpose(2, 1, 0, 3))

    def padk(x, kp):
        x = np.asarray(x, dtype=np.float32)
        if x.shape[0] < kp:
            x = np.concatenate(
                [x, np.zeros((kp - x.shape[0], x.shape[1]), np.float32)], axis=0)
        return np.ascontiguousarray(x)

    def vec128(v, ncol):
        v = np.asarray(v, dtype=np.float32)
        return np.ascontiguousarray(v.reshape(ncol, 128).T)

    p = params
    wrdn = padk(p['W_rd'], K0P)
    kps = [HD, K1P, K2P]
    w1 = [colblocks(p[f'enc{v}']['W1'], DH // 128, kps[v]) for v in range(3)]
    w2a = np.concatenate([colblocks(p[f'enc{v}']['W2'], CD // 128, DH)
                          for v in range(3)], axis=0)
    brd = vec128(p['b_rd'], HD // 128)
    g1a = np.concatenate([vec128(p[f'enc{v}']['g1'], DH // 128)
                          for v in range(3)], axis=1)
    be1a = np.concatenate([vec128(p[f'enc{v}']['be1'], DH // 128)
                           for v in range(3)], axis=1)
    g2a = np.concatenate([vec128(p[f'enc{v}']['g2'], CD // 128)
                          for v in range(3)], axis=1)
    be2a = np.concatenate([vec128(p[f'enc{v}']['be2'], CD // 128)
                           for v in range(3)], axis=1)

    x0 = np.asarray(x0, dtype=np.float32)
    x1 = np.asarray(x1, dtype=np.float32)
    x2 = np.asarray(x2, dtype=np.float32)
    S = np.asarray(S, dtype=np.float32)
    ST = np.ascontiguousarray(S.T)

    in_maps = []
    idx = np.arange(RPC)
    for r in range(NC):
        rows = slice(r * RPC, (r + 1) * RPC)
        sb = np.ascontiguousarray(S[rows, :])
        stb = np.ascontiguousarray(ST[rows, :])
        # diag(S)=1 makes A_ii True so the diagonal contributes ~0 (dist_ii~0)
        sb[idx, r * RPC + idx] = 1.0
        stb[idx, r * RPC + idx] = 1.0
        in_maps.append({
            "x0t": padk(x0[rows].T, K0P),
            "x1t": padk(x1[rows].T, K1P),
            "x2t": padk(x2[rows].T, K2P),
            "wrdn": wrdn, "w1_0": w1[0], "w1_1": w1[1], "w1_2": w1[2],
            "w2a": w2a, "brd": brd, "g1a": g1a, "be1a": be1a,
            "g2a": g2a, "be2a": be2a,
            "sblk": sb, "stblk": stb,
        })
    return in_maps


def kernel(x0, x1, x2, S, batch, params):
    global LAST_RESULT
    nc = _get_nc()
    in_maps = _prep_inmaps(x0, x1, x2, S, params)
    res = run_bass_kernel_spmd(nc, in_maps, list(range(NC)),
                               trace_cores=list(range(NC)))
    LAST_RESULT = res
    fusion = np.concatenate(
        [np.ascontiguousarray(res.results[r]["fusiont"].T) for r in range(NC)],
        axis=0)
    loss = np.float32(sum(float(res.results[r]["lossp"][0, 0])
                          for r in range(NC)))
    return fusion, loss


# revision 27
# speedup vs baseline: 1.3694x; 1.0511x over previous
"""Trainium2 Bass kernel for nn_AdaMus loss_fn (multi-view encoder + pairwise loss).

Strategy: data-parallel over batch (512 rows/core on 8 cores).
 - Activations stored transposed [feature, rows]; all matmuls bf16 with f32 PSUM.
 - Sync-BatchNorm: per-feature sum/sumsq via fused ACT accum, AllReduce'd.
 - fusion (bf16 + its column sumsq row) AllGather'd; pairwise dist row-block
   computed entirely on TensorE via an augmented matmul; loss elementwise on
   DVE/ACT with fused row reductions. Diagonal excluded by setting diag(S)=1
   host-side (constant structural mask) so diag contributes ~0.
 - Outputs per core: fusionT [512,512] f32 + partial loss [1,1]; host gathers.
"""
import sys
if '/opt/trn_rl_repo' not in sys.path:
    sys.path.insert(0, '/opt/trn_rl_repo')

import numpy as np
import concourse.bass as bass
import concourse.mybir as mybir
import concourse.tile as tile
from concourse import bacc
from concourse.bass_utils import run_bass_kernel_spmd

F32 = mybir.dt.float32
F16 = mybir.dt.float16
BF16 = mybir.dt.bfloat16
AF = mybir.ActivationFunctionType
OP = mybir.AluOpType

NC = 8
B = 4096
RPC = B // NC            # 512 rows per core
K0, K1, K2 = 5000, 2000, 1000
K0P, K1P, K2P = 5120, 2048, 1024    # padded to x128 for clean chunking
HD = 1024                # rd_net out
DH = 2048                # encoder hidden
CD = 512                 # com_dim
BN_EPS = 1e-5
LAMBDA1 = 0.01
INV_B = 1.0 / B
SC_G = -2.0 * (INV_B * INV_B)    # -2/B^2, exact power of two
SC_SQ = INV_B * INV_B            # 1/B^2, exact power of two
GSCALE = LAMBDA1 / (CD * NC)     # per-core gamma-L1 term scale

LAST_RESULT = None
_CACHED = {}


def _kchunks(K):
    return [(s, min(128, K - s)) for s in range(0, K, 128)]


def _groups(chs, maxn=8):
    """Group consecutive full chunks into runs of <= maxn; partial chunk alone."""
    out, cur = [], []
    for (s, n) in chs:
        if n == 128 and len(cur) < maxn:
            cur.append((s, n))
        elif n == 128:
            out.append(cur)
            cur = [(s, n)]
        else:
            if cur:
                out.append(cur)
                cur = []
            out.append([(s, n)])
    if cur:
        out.append(cur)
    return out


def _build():
    import os
    STAGE = int(os.environ.get("KSTAGE", "7"))
    nc = bacc.Bacc(None, target_bir_lowering=False, debug=False)

    # ---- I/O declarations -------------------------------------------------
    x0t = nc.dram_tensor("x0t", [K0P, RPC], F32, kind="ExternalInput")
    x1t = nc.dram_tensor("x1t", [K1P, RPC], F32, kind="ExternalInput")
    x2t = nc.dram_tensor("x2t", [K2P, RPC], F32, kind="ExternalInput")
    wrdn = nc.dram_tensor("wrdn", [K0P, HD], F32, kind="ExternalInput")
    w1_0 = nc.dram_tensor("w1_0", [DH // 128, 128, HD // 128, 128], F32, kind="ExternalInput")
    w1_1 = nc.dram_tensor("w1_1", [DH // 128, 128, K1P // 128, 128], F32, kind="ExternalInput")
    w1_2 = nc.dram_tensor("w1_2", [DH // 128, 128, K2P // 128, 128], F32, kind="ExternalInput")
    w2a = nc.dram_tensor("w2a", [3 * (CD // 128), 128, DH // 128, 128], F32, kind="ExternalInput")
    brd = nc.dram_tensor("brd", [128, HD // 128], F32, kind="ExternalInput")
    g1a = nc.dram_tensor("g1a", [128, 3 * (DH // 128)], F32, kind="ExternalInput")
    be1a = nc.dram_tensor("be1a", [128, 3 * (DH // 128)], F32, kind="ExternalInput")
    g2a = nc.dram_tensor("g2a", [128, 3 * (CD // 128)], F32, kind="ExternalInput")
    be2a = nc.dram_tensor("be2a", [128, 3 * (CD // 128)], F32, kind="ExternalInput")
    sblk = nc.dram_tensor("sblk", [RPC, B], F32, kind="ExternalInput")
    stblk = nc.dram_tensor("stblk", [RPC, B], F32, kind="ExternalInput")

    fusiont = nc.dram_tensor("fusiont", [CD, RPC], F32, kind="ExternalOutput")
    lossp = nc.dram_tensor("lossp", [1, 1], F32, kind="ExternalOutput")

    NM1 = DH // 128      # 16 m-chunks for L1
    NM2 = CD // 128      # 4 m-chunks for L2
    NMR = HD // 128      # 8 m-chunks for rd
    RG = [list(range(NC))]

    from contextlib import ExitStack
    with tile.TileContext(nc) as tc, ExitStack() as es:
        # persistent pools
        cp = es.enter_context(tc.tile_pool(name="consts", bufs=1))
        stp = es.enter_context(tc.tile_pool(name="stats", bufs=1))
        fp = es.enter_context(tc.tile_pool(name="fus", bufs=4))
        pp = es.enter_context(tc.tile_pool(name="psum", bufs=8, space="PSUM"))
        dp = es.enter_context(tc.tile_pool(name="dram", bufs=1, space="DRAM"))
        # encoder-phase pools (closed before the loss phase to free SBUF)
        enc_es = ExitStack()
        xp = enc_es.enter_context(tc.tile_pool(name="xh", bufs=24))
        wsp = enc_es.enter_context(tc.tile_pool(name="wstage", bufs=4))
        wbp = enc_es.enter_context(tc.tile_pool(name="wbp", bufs=4))
        xsp = enc_es.enter_context(tc.tile_pool(name="xstage", bufs=4))
        yp = enc_es.enter_context(tc.tile_pool(name="ys", bufs=34))
        ybp = enc_es.enter_context(tc.tile_pool(name="ysb", bufs=12))
        scrp = enc_es.enter_context(tc.tile_pool(name="scr", bufs=2))
        hp = enc_es.enter_context(tc.tile_pool(name="hh", bufs=30))
        vp = enc_es.enter_context(tc.tile_pool(name="v0h", bufs=8))

        # ---- constants + warmup collective ---------------------------
        dum = cp.tile([128, 1], F32, name="dum")
        nc.vector.memset(dum[:], 0.0)
        wu_in = dp.tile([128, 1], F32, name="wu_in")
        wu_out = dp.tile([128, 1], F32, addr_space="Shared", name="wu_out")
        nc.gpsimd.dma_start(wu_in[:], dum[:])
        nc.gpsimd.collective_compute(
            "AllReduce", OP.add, replica_groups=RG,
            ins=[wu_in[:].opt()], outs=[wu_out[:].opt()])

        brd_t = cp.tile([128, NMR], F32, name="brd_t")
        nc.sync.dma_start(brd_t[:], brd[:])
        g1_t = cp.tile([128, 3 * NM1], F32, name="g1_t")
        nc.sync.dma_start(g1_t[:], g1a[:])
        be1_t = cp.tile([128, 3 * NM1], F32, name="be1_t")
        nc.sync.dma_start(be1_t[:], be1a[:])
        g2_t = cp.tile([128, 3 * NM2], F32, name="g2_t")
        nc.sync.dma_start(g2_t[:], g2a[:])
        be2_t = cp.tile([128, 3 * NM2], F32, name="be2_t")
        nc.sync.dma_start(be2_t[:], be2a[:])
        ones_t = cp.tile([128, 1], BF16, name="ones_t")
        nc.vector.memset(ones_t[:], 1.0)
        onesf_t = cp.tile([128, 1], F32, name="onesf_t")
        nc.vector.memset(onesf_t[:], 1.0)
        acc = cp.tile([128, 5], F32, name="acc")
        nc.vector.memset(acc[:], 0.0)
        accw = cp.tile([128, 128], F32, name="accw")
        nc.vector.memset(accw[:], 0.0)
        eps_t = cp.tile([128, 1], F32, name="eps_t")
        nc.vector.memset(eps_t[:], BN_EPS)

        # ---- helpers --------------------------------------------------
        def stage_x(xdram, K, label):
            """DMA + cast input xT chunks -> list of resident bf16 tiles."""
            tiles = []
            for ci, (s, n) in enumerate(_kchunks(K)):
                st = xsp.tile([n, RPC], F32, tag="xstage", name=f"xs_{label}_{ci}")
                nc.sync.dma_start(st[:], xdram[s:s + n, :])
                xb = xp.tile([n, RPC], BF16, tag="xh", name=f"xh_{label}_{ci}")
                nc.vector.tensor_scalar(xb[:], st[:], 1.0, None, OP.mult)
                tiles.append(xb)
            return tiles

        def mm_layer(wdram, KP, n_mc, rhs_tiles, consume, mc_base=0, label=""):
            """m-chunk outer: one full-width weight slab per mc (fat 4-8KB
            per-partition descriptors, 4-way partition-split across DMA
            queues), cast via tensor_scalar (2x mode), matmul-accumulate."""
            nch = KP // 128
            for mc in range(n_mc):
                ps = pp.tile([128, RPC], F32, tag="psum", name=f"ps_{label}_{mc}")
                ws = wsp.tile([128, nch, 128], F32, tag="wstage",
                              name=f"ws_{label}_{mc}")
                qn = min(4, nch)
                cq = nch // qn
                for q in range(qn):
                    nc.sync.dma_start(
                        ws[:, q * cq:(q + 1) * cq, :],
                        wdram[mc_base + mc, :, q * cq:(q + 1) * cq, :])
                wb = wbp.tile([128, nch, 128], BF16, tag="wbp",
                              name=f"wb_{label}_{mc}")
                nc.vector.tensor_scalar(wb[:], ws[:], 1.0, None, OP.mult)
                for ci in range(nch):
                    nc.tensor.matmul(ps[:], wb[:, ci, :], rhs_tiles[ci][:],
                                     start=(ci == 0), stop=(ci == nch - 1))
                consume(mc, ps)

        def l1_consume(stats_t, n_mc, label, ypool):
            ytiles = []
            def consume(mc, ps):
                y = ypool.tile([128, RPC], F16, tag="y", name=f"y_{label}_{mc}")
                nc.scalar.activation(y[:], ps[:], AF.Copy,
                                     accum_out=stats_t[:, mc:mc + 1])
                sc = scrp.tile([128, RPC], F32, tag="scr", name=f"sc_{label}_{mc}")
                nc.scalar.activation(sc[:], ps[:], AF.Square,
                                     accum_out=stats_t[:, n_mc + mc:n_mc + mc + 1])
                ytiles.append(y)
            return consume, ytiles

        def allreduce_stats(stats_t, ncols, label):
            ain = dp.tile([128, ncols], F32, name=f"arin_{label}")
            aout = dp.tile([128, ncols], F32, addr_space="Shared",
                           name=f"arout_{label}")
            nc.gpsimd.dma_start(ain[:], stats_t[:])
            nc.gpsimd.collective_compute(
                "AllReduce", OP.add, replica_groups=RG,
                ins=[ain[:].opt()], outs=[aout[:].opt()])
            back = stp.tile([128, ncols], F32, name=f"arb_{label}")
            nc.gpsimd.dma_start(back[:], aout[:])
            return back

        def ab_math(back, n_mc, g_t, be_t, goff, extra_scale, label):
            """a = g*rsqrt(var+eps)*extra, b = (be - m*g*rsqrt)*extra."""
            m = stp.tile([128, n_mc], F32, name=f"m_{label}")
            nc.vector.tensor_scalar(m[:], back[:, 0:n_mc], INV_B, None, OP.mult)
            e2 = stp.tile([128, n_mc], F32, name=f"e2_{label}")
            nc.vector.tensor_scalar(e2[:], back[:, n_mc:2 * n_mc], INV_B, None,
                                    OP.mult)
            var = stp.tile([128, n_mc], F32, name=f"var_{label}")
            nc.vector.tensor_tensor(var[:], m[:], m[:], OP.mult)
            nc.vector.tensor_tensor(var[:], e2[:], var[:], OP.subtract)
            sd = stp.tile([128, n_mc], F32, name=f"sd_{label}")
            nc.scalar.activation(sd[:], var[:], AF.Sqrt, bias=eps_t[:])
            rs = stp.tile([128, n_mc], F32, name=f"rs_{label}")
            nc.vector.reciprocal(rs[:], sd[:])
            a = stp.tile([128, n_mc], F32, name=f"a_{label}")
            nc.vector.tensor_tensor(a[:], g_t[:, goff:goff + n_mc], rs[:], OP.mult)
            bb = stp.tile([128, n_mc], F32, name=f"b_{label}")
            nc.vector.tensor_tensor(bb[:], m[:], a[:], OP.mult)
            nc.vector.tensor_tensor(bb[:], be_t[:, goff:goff + n_mc], bb[:],
                                    OP.subtract)
            if extra_scale != 1.0:
                nc.vector.tensor_scalar(a[:], a[:], extra_scale, None, OP.mult)
                nc.vector.tensor_scalar(bb[:], bb[:], extra_scale, None, OP.mult)
            return a, bb

        def norm_layer(ytiles, a, bb, n_mc, out_dtype, outpool, label):
            outs = []
            for mc in range(n_mc):
                h = outpool.tile([128, RPC], out_dtype,
                                 tag="hh" if out_dtype == BF16 else "zz",
                                 name=f"h_{label}_{mc}")
                nc.scalar.activation(h[:], ytiles[mc][:], AF.Relu,
                                     bias=bb[:, mc:mc + 1], scale=a[:, mc:mc + 1])
                outs.append(h)
            return outs

        # ================= encoder pipeline ===========================
        with nc.named_scope("L1_enc2"):
            xh2 = stage_x(x2t, K2P, "x2")
            st1_2 = stp.tile([128, 2 * NM1], F32, name="st1_2")
            cons, y2t = l1_consume(st1_2, NM1, "e2l1", yp)
            mm_layer(w1_2, K2P, NM1, xh2, cons, label="e2l1")
        ar2 = allreduce_stats(st1_2, 2 * NM1, "e2l1")

        with nc.named_scope("L1_enc1"):
            xh1 = stage_x(x1t, K1P, "x1")
            st1_1 = stp.tile([128, 2 * NM1], F32, name="st1_1")
            cons, y1t = l1_consume(st1_1, NM1, "e1l1", yp)
            mm_layer(w1_1, K1P, NM1, xh1, cons, label="e1l1")
        ar1 = allreduce_stats(st1_1, 2 * NM1, "e1l1")

        # norm enc2 L1 before rd (AR2 lands during L1_enc1)
        a12, b12 = ab_math(ar2, NM1, g1_t, be1_t, 2 * NM1, 1.0, "e2l1")
        hh2 = norm_layer(y2t, a12, b12, NM1, BF16, hp, "h2")

        # ---- rd net: kc-outer, all 8 psum banks live -----------------
        with nc.named_scope("rd"):
            ps_rd = [pp.tile([128, RPC], F32, tag="psum", name=f"psrd_{mc}")
                     for mc in range(NMR)]
            chs0 = _kchunks(K0P)
            for ci, (s, n) in enumerate(chs0):
                st = xsp.tile([n, RPC], F32, tag="xstage", name=f"xs_x0_{ci}")
                nc.sync.dma_start(st[:], x0t[s:s + n, :])
                xb = xp.tile([n, RPC], BF16, tag="xh", name=f"xh_x0_{ci}")
                nc.vector.tensor_scalar(xb[:], st[:], 1.0, None, OP.mult)
                ws = wsp.tile([n, HD], F32, tag="wstage", name=f"ws_rd_{ci}")
                nc.sync.dma_start(ws[:, 0:HD // 2], wrdn[s:s + n, 0:HD // 2])
                nc.sync.dma_start(ws[:, HD // 2:], wrdn[s:s + n, HD // 2:])
                wb = wbp.tile([n, HD], BF16, tag="wbp", name=f"wb_rd_{ci}")
                nc.vector.tensor_scalar(wb[:], ws[:], 1.0, None, OP.mult)
                for mc in range(NMR):
                    nc.tensor.matmul(ps_rd[mc][:], wb[:, mc * 128:(mc + 1) * 128],
                                     xb[:], start=(ci == 0),
                                     stop=(ci == len(chs0) - 1),
                                     skip_group_check=True)
            v0h = []
            for mc in range(NMR):
                v = vp.tile([128, RPC], BF16, tag="v0", name=f"v0h_{mc}")
                nc.scalar.activation(v[:], ps_rd[mc][:], AF.Relu,
                                     bias=brd_t[:, mc:mc + 1])
                v0h.append(v)

        a11, b11 = ab_math(ar1, NM1, g1_t, be1_t, NM1, 1.0, "e1l1")
        hh1 = norm_layer(y1t, a11, b11, NM1, BF16, hp, "h1")

        with nc.named_scope("L1_enc0"):
            st1_0 = stp.tile([128, 2 * NM1], F32, name="st1_0")
            cons, y0t = l1_consume(st1_0, NM1, "e0l1", yp)
            mm_layer(w1_0, HD, NM1, v0h, cons, label="e0l1")
        ar0 = allreduce_stats(st1_0, 2 * NM1, "e0l1")

        with nc.named_scope("L2_enc2"):
            st2_2 = stp.tile([128, 2 * NM2], F32, name="st2_2")
            cons, y2b = l1_consume(st2_2, NM2, "e2l2", ybp)
            mm_layer(w2a, DH, NM2, hh2, cons, mc_base=2 * NM2, label="e2l2")
        br2 = allreduce_stats(st2_2, 2 * NM2, "e2l2")

        with nc.named_scope("L2_enc1"):
            st2_1 = stp.tile([128, 2 * NM2], F32, name="st2_1")
            cons, y1b = l1_consume(st2_1, NM2, "e1l2", ybp)
            mm_layer(w2a, DH, NM2, hh1, cons, mc_base=NM2, label="e1l2")
        br1 = allreduce_stats(st2_1, 2 * NM2, "e1l2")

        a10, b10 = ab_math(ar0, NM1, g1_t, be1_t, 0, 1.0, "e0l1")
        hh0 = norm_layer(y0t, a10, b10, NM1, BF16, hp, "h0")

        with nc.named_scope("L2_enc0"):
            st2_0 = stp.tile([128, 2 * NM2], F32, name="st2_0")
            cons, y0b = l1_consume(st2_0, NM2, "e0l2", ybp)
            mm_layer(w2a, DH, NM2, hh0, cons, mc_base=0, label="e0l2")
        br0 = allreduce_stats(st2_0, 2 * NM2, "e0l2")

        # ---- norms L2 (scaled by 1/3) + fusion -----------------------
        # z2 normalizes straight into the fusion accumulator tiles
        a22, b22 = ab_math(br2, NM2, g2_t, be2_t, 2 * NM2, 1.0 / 3.0, "e2l2")
        fus = []
        for mc in range(NM2):
            f = fp.tile([128, RPC], F32, tag="fus", name=f"fus_{mc}")
            nc.scalar.activation(f[:], y2b[mc][:], AF.Relu,
                                 bias=b22[:, mc:mc + 1], scale=a22[:, mc:mc + 1])
            fus.append(f)
        a21, b21 = ab_math(br1, NM2, g2_t, be2_t, NM2, 1.0 / 3.0, "e1l2")
        z1 = norm_layer(y1b, a21, b21, NM2, F32, fp, "z1")
        for mc in range(NM2):
            nc.vector.tensor_tensor(fus[mc][:], fus[mc][:], z1[mc][:], OP.add)

        # gamma L1 term -> acc[:,4]  (fills the last-AR wait on ACT)
        gjunk = scrp.tile([128, 3 * NM2], F32, tag="gjunk", name="gjunk")
        nc.scalar.activation(gjunk[:], g2_t[:], AF.Abs, scale=GSCALE,
                             accum_out=acc[:, 4:5])

        a20, b20 = ab_math(br0, NM2, g2_t, be2_t, 0, 1.0 / 3.0, "e0l2")
        z0 = norm_layer(y0b, a20, b20, NM2, F32, fp, "z0")
        for mc in range(NM2):
            nc.vector.tensor_tensor(fus[mc][:], fus[mc][:], z0[mc][:], OP.add)
            nc.sync.dma_start(fusiont[mc * 128:(mc + 1) * 128, :], fus[mc][:])

        # ---- fusion bf16 + sq + AllGather ----------------------------
        with nc.named_scope("gather"):
            fh, fsc, fhsq = [], [], []
            for mc in range(NM2):
                h = fp.tile([128, RPC], BF16, tag="fh", name=f"fh_{mc}")
                nc.vector.tensor_copy(h[:], fus[mc][:])
                fh.append(h)
                s = fp.tile([128, RPC], BF16, tag="fsc", name=f"fsc_{mc}")
                nc.vector.tensor_scalar(s[:], fus[mc][:], SC_G, None, OP.mult)
                fsc.append(s)
                q = fp.tile([128, RPC], BF16, tag="fhsq", name=f"fhsq_{mc}")
                nc.vector.tensor_tensor(q[:], h[:], h[:], OP.mult)
                fhsq.append(q)
            psq = pp.tile([1, RPC], F32, tag="psum", name="psq")
            for mc in range(NM2):
                nc.tensor.matmul(psq[:], ones_t[:], fhsq[mc][:],
                                 start=(mc == 0), stop=(mc == NM2 - 1))
            sqbf = fp.tile([1, RPC], BF16, tag="sqbf", bufs=1, name="sqbf")
            nc.scalar.activation(sqbf[:], psq[:], AF.Copy)
            # lhsT extra rows: row0 = 1/B^2 (for sq_j), row1 = sq_i/B^2
            # (engine ops can't start at partition 1; assemble via DMA)
            c0row = cp.tile([1, RPC], BF16, name="c0row")
            nc.vector.memset(c0row[:], SC_SQ)
            sqsc = cp.tile([1, RPC], BF16, name="sqsc")
            nc.vector.tensor_scalar(sqsc[:], sqbf[:], SC_SQ, None, OP.mult)
            onesrow = cp.tile([1, RPC], BF16, name="onesrow")
            nc.vector.memset(onesrow[:], 1.0)
            lh5 = cp.tile([2, RPC], BF16, name="lh5")
            nc.sync.dma_start(lh5[0:1, :], c0row[:])
            nc.sync.dma_start(lh5[1:2, :], sqsc[:])

            agin = dp.tile([CD + 1, RPC], BF16, name="agin")
            agout = dp.tile([NC * (CD + 1), RPC], BF16, addr_space="Shared",
                            name="agout")
            for mc in range(NM2):
                nc.sync.dma_start(agin[mc * 128:(mc + 1) * 128, :], fh[mc][:])
            nc.sync.dma_start(agin[CD:CD + 1, :], sqbf[:])
            if STAGE >= 4:
                nc.gpsimd.collective_compute(
                    "AllGather", OP.bypass, replica_groups=RG,
                    ins=[agin[:].opt()], outs=[agout[:].opt()])

        # encoder tiles are all dead now; free their SBUF for the loss phase
        enc_es.close()
        gp = es.enter_context(tc.tile_pool(name="gt", bufs=8))
        ssp = es.enter_context(tc.tile_pool(name="sstage", bufs=8))
        lp = es.enter_context(tc.tile_pool(name="ltmp", bufs=3))

        # ---- pairwise dist + loss (A-mask JIT) -----------------------
        with nc.named_scope("loss"):
            spair = {}
            for jc in range(NC if STAGE >= 5 else 0):
                gts = []
                for kc in range(NM2):
                    g = gp.tile([128, RPC], BF16, tag="gt", name=f"g_{jc}_{kc}")
                    nc.sync.dma_start(
                        g[:], agout[jc * (CD + 1) + kc * 128:
                                    jc * (CD + 1) + (kc + 1) * 128, :])
                    gts.append(g)
                sqo = gp.tile([2, RPC], BF16, tag="sqo", bufs=2, name=f"sqo_{jc}")
                nc.sync.dma_start(sqo[0:1, :],
                                  agout[jc * (CD + 1) + CD:
                                        jc * (CD + 1) + CD + 1, :])
                nc.sync.dma_start(sqo[1:2, :], onesrow[:])
                for mc in range(NM2):
                    if jc % 2 == 0:
                        s_t = ssp.tile([128, 2 * RPC], F32, tag="sstage",
                                       name=f"s_{jc}_{mc}")
                        nc.sync.dma_start(
                            s_t[:], sblk[mc * 128:(mc + 1) * 128,
                                         jc * RPC:(jc + 2) * RPC])
                        st_t = ssp.tile([128, 2 * RPC], F32, tag="ststage",
                                        name=f"st_{jc}_{mc}")
                        nc.sync.dma_start(
                            st_t[:], stblk[mc * 128:(mc + 1) * 128,
                                           jc * RPC:(jc + 2) * RPC])
                        spair[mc] = (s_t, st_t)
                    s_t, st_t = spair[mc]
                    off = (jc % 2) * RPC
                    mx = lp.tile([128, RPC], F32, tag="mx", name=f"mx_{jc}_{mc}")
                    nc.vector.tensor_tensor(mx[:], s_t[:, off:off + RPC],
                                            st_t[:, off:off + RPC], OP.max)
                    at = lp.tile([128, RPC], F16, tag="at", name=f"at_{jc}_{mc}")
                    nc.vector.tensor_scalar(at[:], mx[:], 0.6, None, OP.is_gt)

                    ps = pp.tile([128, RPC], F32, tag="psum",
                                 name=f"pl_{jc}_{mc}")
                    for kc in range(NM2):
                        nc.tensor.matmul(
                            ps[:], fsc[kc][:, mc * 128:(mc + 1) * 128],
                            gts[kc][:], start=(kc == 0), stop=False)
                    nc.tensor.matmul(ps[:], lh5[:, mc * 128:(mc + 1) * 128],
                                     sqo[:], start=False, stop=True)
                    # Algebra (valid since d<<1 so relu(1-d)=1-d exactly):
                    #   sum[neg + at*(dist-neg)] =
                    #   N - 2*sum(d) + sum(dist) + 2*sum(at*d) - sum(at)
                    ci4 = jc * NM2 + mc
                    # pc = clamped dist bf16; d = sqrt(pc) f16 with fused sum(d)
                    pc = lp.tile([128, RPC], BF16, tag="pc", name=f"pc_{jc}_{mc}")
                    nc.scalar.activation(pc[:], ps[:], AF.Relu)
                    d = lp.tile([128, RPC], F16, tag="u1", name=f"d_{jc}_{mc}")
                    nc.scalar.activation(d[:], pc[:], AF.Sqrt,
                                         accum_out=accw[:, ci4:ci4 + 1])
                    if STAGE >= 6:
                        # sum(dist) from psum (f32) with fused accum
                        dco = lp.tile([128, RPC], BF16, tag="neg",
                                      name=f"dco_{jc}_{mc}")
                        nc.scalar.activation(dco[:], ps[:], AF.Copy,
                                             accum_out=accw[:, 32 + ci4:33 + ci4])
                    if STAGE >= 7:
                        # sum(at*d) and sum(at)
                        ad = lp.tile([128, RPC], F16, tag="adn",
                                     name=f"ad_{jc}_{mc}")
                        nc.vector.tensor_tensor(ad[:], at[:], d[:], OP.mult)
                        nc.vector.tensor_reduce(accw[:, 64 + ci4:65 + ci4], ad[:],
                                                mybir.AxisListType.X, OP.add)
                        nc.vector.tensor_reduce(accw[:, 96 + ci4:97 + ci4], at[:],
                                                mybir.AxisListType.X, OP.add)

            # finalize: columns [0:32]=sum(d), [32:64]=sum(dist),
            # [64:96]=sum(at*d), [96:128]=sum(at); coefficients -2,+1,+2,-1;
            # plus the constant N_total = B*RPC per core.
            if STAGE >= 7:
                nc.vector.tensor_scalar(accw[:, 0:32], accw[:, 0:32], -2.0,
                                        None, OP.mult)
                nc.vector.tensor_scalar(accw[:, 64:96], accw[:, 64:96], 2.0,
                                        None, OP.mult)
                nc.vector.tensor_scalar(accw[:, 96:128], accw[:, 96:128], -1.0,
                                        None, OP.mult)
                nc.vector.tensor_reduce(acc[:, 0:1], accw[:],
                                        mybir.AxisListType.X, OP.add)
                # += N_total/128 per partition (rows per partition sum to B*RPC)
                nc.vector.tensor_scalar(acc[:, 0:1], acc[:, 0:1],
                                        float(B) * RPC / 128.0, None, OP.add)
                nc.vector.tensor_scalar(acc[:, 0:4], acc[:, 0:4], 0.5, None,
                                        OP.mult)

            ps5 = pp.tile([1, 5], F32, tag="psum", name="ps5")
            nc.tensor.matmul(ps5[:], onesf_t[:], acc[:], start=True, stop=True)
            rr = cp.tile([1, 1], F32, name="rr")
            nc.vector.tensor_reduce(rr[:], ps5[:], mybir.AxisListType.X, OP.add)
            nc.sync.dma_start(lossp[:], rr[:])

    nc.compile()
    return nc


def _get_nc():
    if "nc" not in _CACHED:
        _CACHED["nc"] = _build()
    return _CACHED["nc"]


def _prep_inmaps(x0, x1, x2, S, params):
    def colblocks(W, nm, kp):
        """-> [nm, 128p, kp//128, 128j], 4KB-contiguous per partition."""
        W = np.asarray(W, dtype=np.float32)
        din, dout = W.shape
        if din < kp:
            W = np.concatenate(
                [W, np.zeros((kp - din, dout), np.float32)], axis=0)
        # [kp,dout] -> [nch,128p,nm,128j] -> [nm,128p,nch,128j]
        return np.ascontiguousarray(
            W.reshape(kp // 128, 128, nm, 128).transpose(2, 1, 0, 3))

    def padk(x, kp):
        x = np.asarray(x, dtype=np.float32)
        if x.shape[0] < kp:
            x = np.concatenate(
                [x, np.zeros((kp - x.shape[0], x.shape[1]), np.float32)], axis=0)
        return np.ascontiguousarray(x)

    def vec128(v, ncol):
        v = np.asarray(v, dtype=np.float32)
        return np.ascontiguousarray(v.reshape(ncol, 128).T)

    p = params
    wrdn = padk(p['W_rd'], K0P)
    kps = [HD, K1P, K2P]
    w1 = [colblocks(p[f'enc{v}']['W1'], DH // 128, kps[v]) for v in range(3)]
    w2a = np.concatenate([colblocks(p[f'enc{v}']['W2'], CD // 128, DH)
                          for v in range(3)], axis=0)
    brd = vec128(p['b_rd'], HD // 128)
    g1a = np.concatenate([vec128(p[f'enc{v}']['g1'], DH // 128)
                          for v in range(3)], axis=1)
    be1a = np.concatenate([vec128(p[f'enc{v}']['be1'], DH // 128)
                           for v in range(3)], axis=1)
    g2a = np.concatenate([vec128(p[f'enc{v}']['g2'], CD // 128)
                          for v in range(3)], axis=1)
    be2a = np.concatenate([vec128(p[f'enc{v}']['be2'], CD // 128)
                           for v in range(3)], axis=1)

    x0 = np.asarray(x0, dtype=np.float32)
    x1 = np.asarray(x1, dtype=np.float32)
    x2 = np.asarray(x2, dtype=np.float32)
    S = np.asarray(S, dtype=np.float32)
    ST = np.ascontiguousarray(S.T)

    in_maps = []
    idx = np.arange(RPC)
    for r in range(NC):
        rows = slice(r * RPC, (r + 1) * RPC)
        sb = np.ascontiguousarray(S[rows, :])
        stb = np.ascontiguousarray(ST[rows, :])
        # diag(S)=1 makes A_ii True so the diagonal contributes ~0 (dist_ii~0)
        sb[idx, r * RPC + idx] = 1.0
        stb[idx, r * RPC + idx] = 1.0
        in_maps.append({
            "x0t": padk(x0[rows].T, K0P),
            "x1t": padk(x1[rows].T, K1P),
            "x2t": padk(x2[rows].T, K2P),
            "wrdn": wrdn, "w1_0": w1[0], "w1_1": w1[1], "w1_2": w1[2],
            "w2a": w2a, "brd": brd, "g1a": g1a, "be1a": be1a,
            "g2a": g2a, "be2a": be2a,
            "sblk": sb, "stblk": stb,
        })
    return in_maps


def kernel(x0, x1, x2, S, batch, params):
    global LAST_RESULT
    nc = _get_nc()
    in_maps = _prep_inmaps(x0, x1, x2, S, params)
    res = run_bass_kernel_spmd(nc, in_maps, list(range(NC)),
                               trace_cores=list(range(NC)))
    LAST_RESULT = res
    fusion = np.concatenate(
        [np.ascontiguousarray(res.results[r]["fusiont"].T) for r in range(NC)],
        axis=0)
    loss = np.float32(sum(float(res.results[r]["lossp"][0, 0])
                          for r in range(NC)))
    return fusion, loss
